# revision 12
# baseline (speedup 1.0000x reference)
"""Trainium2 Bass kernel: 3x3 valid conv (64ch -> 128ch) + per-pixel bias.

Strategy: shard the 510 output rows spatially across 8 NeuronCores (64
rows/core with a 2-row input halo; core 7 overlaps core 6 by 2 rows).
Inside a core, the 64-row band is split across the two PE row-strips:
partitions 0-63 hold the input rows for output rows 0-31 of the band,
partitions 64-127 the rows for output rows 32-63 (the host feeds the
band pre-split so every DMA runs at full 128-partition width).  Each
output row is 9 accumulating K=64 fp16 matmuls (one per kernel tap,
N=510); the two strips run concurrently on the PE halves, so a
tap-pair costs one N=510 stream (~215 ns) and the matmul stream is the
hard floor (~62 us).  Bias is added during PSUM evacuation on the
Vector engine.

Everything rides as fp16 (x, w, bias, y; PSUM accumulates fp32).  The
schedule keeps the PE stream gap-free:
 - head: the full weight tile rides the sync HWDGE ring while the
   first three input rows ride the scalar ring, so the first real
   matmul issues ~2 us into the measured window; the remaining input
   band streams on the gpsimd SWDGE ring, which otherwise sits idle,
   leaving both HWDGE rings to the bias/output streams.
 - warm-up: ~26 short matmuls on a zeroed scratch tile ramp the PE out
   of its low-power state while the head DMAs fly, ending right as the
   real data lands so the stream starts at full clock with no gap.
 - steady state: bias loads and output stores are merged into one
   1 MB two-window DMA per 4-row group (both strips in one transfer)
   and alternate between the two HWDGE rings; 4 PSUM banks per strip
   give the Vector evacuation ~8 us of slack so it never back-
   pressures the PE.
 - tail: the last group stores per row-pair, alternating rings, so the
   final drain after the last matmul is ~2 us.
fp16 keeps 10 mantissa bits; pipeline error vs the fp32 reference is
~5e-4.
"""

import numpy as np
from contextlib import ExitStack

import concourse.bass as bass
import concourse.tile as tile
from concourse import bacc, mybir
from concourse import bass_utils

C, H, W = 64, 512, 512
D, KK = 128, 3
OH, OW = H - KK + 1, W - KK + 1          # 510, 510
NCORES = 8
RPC = 64                                  # output rows per core
BAND = RPC + KK - 1                       # 66 input rows per core
HALF = RPC // 2                           # 32 output rows per strip
IBAND = HALF + KK - 1                     # 34 input rows per strip
GROUPS = 8
GROWS = HALF // GROUPS                    # 4 pair-rows per group
WARMUPS = 30

f32 = mybir.dt.float32
f16 = mybir.dt.float16

# row offset of each core's output band
STARTS = [min(i * RPC, OH - RPC) for i in range(NCORES)]

_CACHE = {}

# results of the last hardware run (inspected by test harnesses)
LAST_RESULTS = None


def _build_program():
    nc = bacc.Bacc(
        "TRN2", target_bir_lowering=False, debug=False, num_devices=NCORES
    )
    # x is pre-split on the host: row (h*64+c) holds band rows
    # [32h, 32h+34) of channel c, flattened
    x = nc.dram_tensor("x", [2 * C, IBAND * W], f16, kind="ExternalInput").ap()
    # w is pre-duplicated: rows 0-63 and 64-127 identical, [c, (ky kx d)]
    w = nc.dram_tensor("w", [2 * C, 9 * D], f16, kind="ExternalInput").ap()
    b = nc.dram_tensor("b", [D, RPC, OW], f16, kind="ExternalInput").ap()
    y = nc.dram_tensor("y", [D, RPC, OW], f16, kind="ExternalOutput").ap()

    # two-window views: rows split into the two strips so one DMA can
    # carry both strips' rows of a group
    b2 = b.rearrange("d (s r) x -> d s (r x)", s=2)   # [D, 2, HALF*OW]
    y2 = y.rearrange("d (s r) x -> d s (r x)", s=2)

    with tile.TileContext(nc) as tc:
        with ExitStack() as ctx:
            xp = ctx.enter_context(tc.tile_pool(name="xin", bufs=1))
            wp = ctx.enter_context(tc.tile_pool(name="wt", bufs=1))
            bp = ctx.enter_context(tc.tile_pool(name="bias", bufs=5))
            op = ctx.enter_context(tc.tile_pool(name="out", bufs=6))
            pp = ctx.enter_context(tc.tile_pool(name="ps", bufs=4, space="PSUM"))
            sp = ctx.enter_context(tc.tile_pool(name="scr", bufs=1))

            wt = wp.tile([128, 9 * D], f16)
            xin = xp.tile([128, IBAND * W], f16)
            warm = sp.tile([128, 256], f16)

            nc.vector.memset(warm[:], 0.0)

            def load_bias(g, eng):
                t = bp.tile([128, 2 * GROWS * OW], f16, tag="bg")
                tv = t[:].rearrange("p (s n) -> p s n", s=2)
                sl = slice(g * GROWS * OW, (g + 1) * GROWS * OW)
                eng.dma_start(tv, b2[:, :, sl])
                return t

            def load_x(r0, r1, eng):
                eng.dma_start(xin[:, r0 * W:r1 * W], x[:, r0 * W:r1 * W])

            # head: strictly deadline-ordered.  The stream-critical
            # bytes (weights + input rows 0-2) are split three ways —
            # both HWDGE rings plus one small gpsimd SWDGE piece — so
            # they land ~2 us after the queues go live; the remaining
            # input rows and the bias groups interleave behind them so
            # everything arrives just ahead of its first use under the
            # shared ~300 GB/s early-phase HBM budget.
            nc.sync.dma_start(wt[:, 0:5 * D], w[:, 0:5 * D])
            load_x(0, 1, nc.sync)                   # row 0
            load_x(1, 2, nc.scalar)                 # row 1
            nc.scalar.dma_start(wt[:, 5 * D:9 * D], w[:, 5 * D:9 * D])
            load_x(2, 3, nc.scalar)                 # row 2
            load_x(3, 4, nc.sync)                   # row 3
            load_x(4, 5, nc.sync)                   # row 4
            load_x(5, 6, nc.scalar)                 # row 5
            bias_tiles = {0: load_bias(0, nc.sync)}
            load_x(6, 10, nc.scalar)                # rows 6-9
            load_x(10, 16, nc.sync)                 # rows 10-15
            bias_tiles[1] = load_bias(1, nc.scalar)
            bias_tiles[2] = load_bias(2, nc.sync)
            bias_tiles[3] = load_bias(3, nc.scalar)
            load_x(16, 24, nc.sync)                 # rows 16-23
            load_x(24, IBAND, nc.scalar)            # rows 24-33

            # PE warm-up: short dummy matmuls on the zeroed scratch tile
            # ramp the array to full clock while the head DMAs land
            pwarm = pp.tile([128, OW], f32, tag="pa")
            for _ in range(WARMUPS):
                nc.tensor.matmul(
                    pwarm[:, 0:128],
                    warm[0:64, 0:128],
                    warm[0:64, 128:256],
                    start=True, stop=True,
                )

            for g in range(GROUPS):
                ra = g * GROWS                 # strip-local first row
                if g + 4 < GROUPS:
                    eng = nc.sync if (g + 4) % 2 == 0 else nc.scalar
                    bias_tiles[g + 4] = load_bias(g + 4, eng)
                bt = bias_tiles.pop(g)
                yg = op.tile([128, 2 * GROWS * OW], f16, tag="yg")
                yv = yg[:].rearrange("p (s n) -> p s n", s=2)

                # tap-outer microgroups of 2 pair-rows: within a
                # microgroup each tap's weights load once per PE half
                # and feed both rows' matmuls (the redundant ldweights
                # are stripped after tracing), cutting the PE-queue
                # issue work ~40% so the stream runs at the pure
                # 215 ns/pair floor
                for m in range(GROWS // 2):
                    j0 = 2 * m
                    pas = [pp.tile([128, OW], f32, tag="pa", name=f"pa{u}")
                           for u in range(2)]
                    pbs = [pp.tile([128, OW], f32, tag="pb", name=f"pb{u}")
                           for u in range(2)]
                    for t in range(9):
                        ky, kx = divmod(t, 3)
                        for u in range(2):
                            off = (ra + j0 + u + ky) * W + kx
                            nc.tensor.matmul(
                                pas[u][:],
                                wt[0:64, t * D:(t + 1) * D],
                                xin[0:64, off:off + OW],
                                start=(t == 0), stop=(t == 8),
                            )
                            nc.tensor.matmul(
                                pbs[u][:],
                                wt[64:128, t * D:(t + 1) * D],
                                xin[64:128, off:off + OW],
                                start=(t == 0), stop=(t == 8),
                            )
                    for u in range(2):
                        j = j0 + u
                        sa = slice(j * OW, (j + 1) * OW)
                        sb = slice((GROWS + j) * OW, (GROWS + j + 1) * OW)
                        nc.vector.tensor_add(yg[:, sa], pas[u][:], bt[:, sa])
                        nc.vector.tensor_add(yg[:, sb], pbs[u][:], bt[:, sb])
                        if g == GROUPS - 1:
                            # tail: per-row per-strip stores on separate
                            # rings so each row drains as soon as its
                            # own bias-add lands
                            nc.sync.dma_start(
                                y2[:, 0:1, (ra + j) * OW:(ra + j + 1) * OW],
                                yv[:, 0:1, j * OW:(j + 1) * OW],
                            )
                            nc.scalar.dma_start(
                                y2[:, 1:2, (ra + j) * OW:(ra + j + 1) * OW],
                                yv[:, 1:2, j * OW:(j + 1) * OW],
                            )

                if g < GROUPS - 1:
                    eng = nc.sync if g % 2 == 0 else nc.scalar
                    eng.dma_start(
                        y2[:, :, ra * OW:(ra + GROWS) * OW], yv[:]
                    )

    # strip ldweights that reload the exact weights already resident in
    # the same PE-half (tile_position): consecutive same-tap matmuls in
    # a microgroup and the warm-up run reuse the loaded array state, so
    # only the first load of each run is kept
    for bb in nc.main_func.blocks:
        last_sig = {}
        for ins in list(bb.instructions):
            if isinstance(ins, mybir.InstLdweights):
                pos = str(ins.tile_position)
                sig = str(ins.ins[0])
                if last_sig.get(pos) == sig:
                    bb.instructions.remove(ins)
                else:
                    last_sig[pos] = sig

    nc.compile()
    return nc


def kernel(input, kernels, biases):
    global LAST_RESULTS
    if "nc" not in _CACHE:
        _CACHE["nc"] = _build_program()
    nc = _CACHE["nc"]

    xh = np.ascontiguousarray(input).astype(np.float16)        # [C, H, W]
    w1 = np.ascontiguousarray(
        kernels.transpose(1, 2, 3, 0)
    ).reshape(C, 9 * D).astype(np.float16)
    wr = np.concatenate([w1, w1], axis=0)                      # [128, 9*D]
    bh = np.ascontiguousarray(biases).astype(np.float16)

    in_maps = []
    for s in STARTS:
        band = xh[:, s:s + BAND, :]
        xs = np.concatenate(
            [band[:, 0:IBAND, :], band[:, HALF:HALF + IBAND, :]], axis=0
        ).reshape(2 * C, IBAND * W)
        in_maps.append({
            "x": np.ascontiguousarray(xs),
            "w": wr,
            "b": np.ascontiguousarray(bh[:, s:s + RPC, :]),
        })

    res = bass_utils.run_bass_kernel_spmd(
        nc, in_maps, core_ids=list(range(NCORES))
    )
    LAST_RESULTS = res

    out = np.empty((D, OH, OW), np.float32)
    for i, s in enumerate(STARTS):
        out[:, s:s + RPC, :] = res.results[i]["y"].astype(np.float32)
    return out


# revision 13
# speedup vs baseline: 1.0192x; 1.0192x over previous
"""Trainium2 Bass kernel: 3x3 valid conv (64ch -> 128ch) + per-pixel bias.

Strategy: shard the 510 output rows spatially across 8 NeuronCores (64
rows/core with a 2-row input halo; core 7 overlaps core 6 by 2 rows).
Inside a core, the 64-row band is split across the two PE row-strips:
partitions 0-63 hold the input rows for output rows 0-31 of the band,
partitions 64-127 the rows for output rows 32-63 (the host feeds the
band pre-split so every DMA runs at full 128-partition width).  Each
output row is 9 accumulating K=64 fp16 matmuls (one per kernel tap,
N=510); the two strips run concurrently on the PE halves, so a
tap-pair costs one N=510 stream (~215 ns) and the matmul stream is the
hard floor (~62 us).  Bias is added during PSUM evacuation on the
Vector engine.

Everything rides as fp16 (x, w, bias, y; PSUM accumulates fp32).  The
schedule keeps the PE stream gap-free:
 - head: the full weight tile rides the sync HWDGE ring while the
   first three input rows ride the scalar ring, so the first real
   matmul issues ~2 us into the measured window; the remaining input
   band streams on the gpsimd SWDGE ring, which otherwise sits idle,
   leaving both HWDGE rings to the bias/output streams.
 - warm-up: ~26 short matmuls on a zeroed scratch tile ramp the PE out
   of its low-power state while the head DMAs fly, ending right as the
   real data lands so the stream starts at full clock with no gap.
 - steady state: bias loads and output stores are merged into one
   1 MB two-window DMA per 4-row group (both strips in one transfer)
   and alternate between the two HWDGE rings; 4 PSUM banks per strip
   give the Vector evacuation ~8 us of slack so it never back-
   pressures the PE.
 - tail: the last group stores per row-pair, alternating rings, so the
   final drain after the last matmul is ~2 us.
fp16 keeps 10 mantissa bits; pipeline error vs the fp32 reference is
~5e-4.
"""

import numpy as np
from contextlib import ExitStack

import concourse.bass as bass
import concourse.tile as tile
from concourse import bacc, mybir
from concourse import bass_utils

C, H, W = 64, 512, 512
D, KK = 128, 3
OH, OW = H - KK + 1, W - KK + 1          # 510, 510
NCORES = 8
RPC = 64                                  # output rows per core
BAND = RPC + KK - 1                       # 66 input rows per core
HALF = RPC // 2                           # 32 output rows per strip
IBAND = HALF + KK - 1                     # 34 input rows per strip
GROUPS = 8
GROWS = HALF // GROUPS                    # 4 pair-rows per group
WARMUPS = 30

f32 = mybir.dt.float32
f16 = mybir.dt.float16

# row offset of each core's output band
STARTS = [min(i * RPC, OH - RPC) for i in range(NCORES)]

_CACHE = {}

# results of the last hardware run (inspected by test harnesses)
LAST_RESULTS = None


def _build_program():
    nc = bacc.Bacc(
        "TRN2", target_bir_lowering=False, debug=False, num_devices=NCORES
    )
    # x is pre-split on the host: row (h*64+c) holds band rows
    # [32h, 32h+34) of channel c, flattened
    x = nc.dram_tensor("x", [2 * C, IBAND * W], f16, kind="ExternalInput").ap()
    # w is pre-duplicated: rows 0-63 and 64-127 identical, [c, (ky kx d)]
    w = nc.dram_tensor("w", [2 * C, 9 * D], f16, kind="ExternalInput").ap()
    b = nc.dram_tensor("b", [D, RPC, OW], f16, kind="ExternalInput").ap()
    y = nc.dram_tensor("y", [D, RPC, OW], f16, kind="ExternalOutput").ap()

    # two-window views: rows split into the two strips so one DMA can
    # carry both strips' rows of a group
    b2 = b.rearrange("d (s r) x -> d s (r x)", s=2)   # [D, 2, HALF*OW]
    y2 = y.rearrange("d (s r) x -> d s (r x)", s=2)

    with tile.TileContext(nc) as tc:
        with ExitStack() as ctx:
            xp = ctx.enter_context(tc.tile_pool(name="xin", bufs=1))
            wp = ctx.enter_context(tc.tile_pool(name="wt", bufs=1))
            bp = ctx.enter_context(tc.tile_pool(name="bias", bufs=5))
            op = ctx.enter_context(tc.tile_pool(name="out", bufs=6))
            pp = ctx.enter_context(tc.tile_pool(name="ps", bufs=4, space="PSUM"))
            sp = ctx.enter_context(tc.tile_pool(name="scr", bufs=1))

            wt = wp.tile([128, 9 * D], f16)
            xin = xp.tile([128, IBAND * W], f16)
            warm = sp.tile([128, 256], f16)

            nc.vector.memset(warm[:], 0.0)

            def load_bias(g, eng):
                t = bp.tile([128, 2 * GROWS * OW], f16, tag="bg")
                tv = t[:].rearrange("p (s n) -> p s n", s=2)
                sl = slice(g * GROWS * OW, (g + 1) * GROWS * OW)
                eng.dma_start(tv, b2[:, :, sl])
                return t

            def load_x(r0, r1, eng):
                eng.dma_start(xin[:, r0 * W:r1 * W], x[:, r0 * W:r1 * W])

            # head: strictly deadline-ordered.  The stream-critical
            # bytes (weights + input rows 0-2) are split three ways —
            # both HWDGE rings plus one small gpsimd SWDGE piece — so
            # they land ~2 us after the queues go live; the remaining
            # input rows and the bias groups interleave behind them so
            # everything arrives just ahead of its first use under the
            # shared ~300 GB/s early-phase HBM budget.
            nc.sync.dma_start(wt[:, 0:5 * D], w[:, 0:5 * D])
            load_x(0, 1, nc.sync)                   # row 0
            load_x(1, 2, nc.scalar)                 # row 1
            nc.scalar.dma_start(wt[:, 5 * D:9 * D], w[:, 5 * D:9 * D])
            load_x(2, 3, nc.scalar)                 # row 2
            load_x(3, 4, nc.sync)                   # row 3
            load_x(4, 5, nc.sync)                   # row 4
            load_x(5, 6, nc.scalar)                 # row 5
            bias_tiles = {0: load_bias(0, nc.sync)}
            load_x(6, 10, nc.scalar)                # rows 6-9
            load_x(10, 16, nc.sync)                 # rows 10-15
            bias_tiles[1] = load_bias(1, nc.scalar)
            bias_tiles[2] = load_bias(2, nc.sync)
            bias_tiles[3] = load_bias(3, nc.scalar)
            load_x(16, 24, nc.sync)                 # rows 16-23
            load_x(24, IBAND, nc.scalar)            # rows 24-33

            # PE warm-up: short dummy matmuls on the zeroed scratch tile
            # ramp the array to full clock while the head DMAs land
            pwarm = pp.tile([128, OW], f32, tag="pa")
            for _ in range(WARMUPS):
                nc.tensor.matmul(
                    pwarm[:, 0:128],
                    warm[0:64, 0:128],
                    warm[0:64, 128:256],
                    start=True, stop=True,
                )

            for g in range(GROUPS):
                ra = g * GROWS                 # strip-local first row
                if g + 4 < GROUPS:
                    eng = nc.sync if (g + 4) % 2 == 0 else nc.scalar
                    bias_tiles[g + 4] = load_bias(g + 4, eng)
                bt = bias_tiles.pop(g)
                yg = op.tile([128, 2 * GROWS * OW], f16, tag="yg")
                yv = yg[:].rearrange("p (s n) -> p s n", s=2)

                for j in range(GROWS):
                    yl = ra + j                # strip-local output row
                    pa = pp.tile([128, OW], f32, tag="pa")
                    pb = pp.tile([128, OW], f32, tag="pb")
                    for t in range(9):
                        ky, kx = divmod(t, 3)
                        off = (yl + ky) * W + kx
                        nc.tensor.matmul(
                            pa[:],
                            wt[0:64, t * D:(t + 1) * D],
                            xin[0:64, off:off + OW],
                            start=(t == 0), stop=(t == 8),
                        )
                        nc.tensor.matmul(
                            pb[:],
                            wt[64:128, t * D:(t + 1) * D],
                            xin[64:128, off:off + OW],
                            start=(t == 0), stop=(t == 8),
                        )
                    sa = slice(j * OW, (j + 1) * OW)
                    sb = slice((GROWS + j) * OW, (GROWS + j + 1) * OW)
                    nc.vector.tensor_add(yg[:, sa], pa[:], bt[:, sa])
                    nc.vector.tensor_add(yg[:, sb], pb[:], bt[:, sb])
                    if g == GROUPS - 1:
                        # tail: per-row per-strip stores on separate
                        # rings so each row drains as soon as its own
                        # bias-add lands (strip a doesn't wait for b)
                        nc.sync.dma_start(
                            y2[:, 0:1, (ra + j) * OW:(ra + j + 1) * OW],
                            yv[:, 0:1, j * OW:(j + 1) * OW],
                        )
                        nc.scalar.dma_start(
                            y2[:, 1:2, (ra + j) * OW:(ra + j + 1) * OW],
                            yv[:, 1:2, j * OW:(j + 1) * OW],
                        )

                if g < GROUPS - 1:
                    eng = nc.sync if g % 2 == 0 else nc.scalar
                    eng.dma_start(
                        y2[:, :, ra * OW:(ra + GROWS) * OW], yv[:]
                    )

    nc.compile()
    return nc


def kernel(input, kernels, biases):
    global LAST_RESULTS
    if "nc" not in _CACHE:
        _CACHE["nc"] = _build_program()
    nc = _CACHE["nc"]

    xh = np.ascontiguousarray(input).astype(np.float16)        # [C, H, W]
    w1 = np.ascontiguousarray(
        kernels.transpose(1, 2, 3, 0)
    ).reshape(C, 9 * D).astype(np.float16)
    wr = np.concatenate([w1, w1], axis=0)                      # [128, 9*D]
    bh = np.ascontiguousarray(biases).astype(np.float16)

    in_maps = []
    for s in STARTS:
        band = xh[:, s:s + BAND, :]
        xs = np.concatenate(
            [band[:, 0:IBAND, :], band[:, HALF:HALF + IBAND, :]], axis=0
        ).reshape(2 * C, IBAND * W)
        in_maps.append({
            "x": np.ascontiguousarray(xs),
            "w": wr,
            "b": np.ascontiguousarray(bh[:, s:s + RPC, :]),
        })

    res = bass_utils.run_bass_kernel_spmd(
        nc, in_maps, core_ids=list(range(NCORES))
    )
    LAST_RESULTS = res

    out = np.empty((D, OH, OW), np.float32)
    for i, s in enumerate(STARTS):
        out[:, s:s + RPC, :] = res.results[i]["y"].astype(np.float32)
    return out
